# revision 9
# baseline (speedup 1.0000x reference)
"""Chamfer distance kernel for Trainium2 (8 NeuronCores, Bass/Tile).

Problem: B=4 pairs of 3-D point clouds with N=8192 points each.
  gt_pc  = coords + registration_gt   (rows  i of the distance matrix)
  gen_pc = coords + registration_pred (cols  j of the distance matrix)
  out = mean_b sum_i min_j d2[b,i,j] + mean_b sum_j min_i d2[b,i,j]

Strategy
  - Sharding: 8 cores = 4 batches x 2 column-halves (all 8192 rows, 4096
    cols each). Col-mins complete per core; row-min partials are
    min-combined across the 2 sibling cores on the host.
  - On-device: one augmented K=15 bf16 matmul produces squared distances
    in PSUM fp32 (hi/lo-split bf16 coordinates; exact in the PE's fp32
    accumulator up to the dropped xl.yl term, ~1e-5 relative).
  - The old kernel pushed every distance element through the DVE once at
    1 elem/cycle (fused custom op, 294us). This version splits the work
    so DVE and ACT are BOTH saturated:
      * alpha stripe (128 of every 2048 cols): DVE custom fused op reads
        PSUM fp32 directly -- col-min + row-min in one 1x pass.
      * staged stripe (1920 cols/group): ACT copies PSUM->SBUF with
        fp32->bf16 downcast (1 elem/cycle on the otherwise-idle ACT);
        then the DVE runs in its 16-bit high-perf modes on the bf16 data:
        tensor_tensor min col-fold at 2x (2x_1p) and a tensor_scalar
        (op0=min vs +BIG, op1=min accum_out) row-min reduce at 4x (4x_2p).
    Per i-block (2 groups x 2048 cols): DVE = 2x258 + 2060 + 1060 =
    3636 ns, ACT = 2x1785 = 3570 ns -- balanced. 64 i-blocks = 233 us
    vs the old all-DVE 281 us. (GPSIMD/compute-DMA/fp8 paths were probed
    and rejected: TensorTensor does not encode on Pool, DMACopy supports
    neither min nor max in Copy or CCE mode.)
  - bf16 distances cost ~0.4% relative per element; the final scalar is a
    sum of mins, measured ~1e-3 relative error -- tolerance is 2e-2.
  - Tail: colacc partition-min via PE transposes (bf16 ident for the bf16
    colacc) packed 4 per PSUM buffer + batched free-axis min reduces.
"""

import numpy as np

import concourse.bass as bass
import concourse.mybir as mybir
from concourse import bacc
from concourse import dve_ops as _dve_ops
from concourse.dve_spec import Spec, Src0, Src1, C0, minn, lower as _dve_lower
from concourse.dve_uop import AluInp, DveOpSpec
from concourse.dve_table_gen import free_opcode_rows
from concourse.tile import TileContext
from concourse.bass_utils import run_bass_kernel_spmd

B = 4
N = 8192
N_CORES = 8
COLS = N // 2            # columns per core (column-sharded: all rows local)
IB = N // 128            # 64 i-blocks per core
GW = 2048                # PSUM group width (4 banks)
JG = COLS // GW          # 2 j-groups
NQ = GW // 512           # 4 matmuls per group
F32 = mybir.dt.float32
BF16 = mybir.dt.bfloat16
KA = 15                  # augmented contraction dim (bf16 hi/lo split)
BIG = 3.0e38
WA = 128                 # alpha stripe width per group (custom fused, fp32)
WS = GW - WA             # staged stripe width per group (bf16 2x/4x path)
SW = JG * WS             # staged width per i-block (one TT + one TS op)
ACHUNKS = JG * WA // 128         # colacc_a transpose chunks (2)
SCHUNKS = SW // 128              # colacc_s transpose chunks (30)
NCHUNK = ACHUNKS + SCHUNKS       # 32
OUTW = IB + IB + NCHUNK          # rm_a | rm_s | colminT

# set by test harness to collect a profile
TRACE = False
LAST_RESULTS = None

_NC_CACHE = None


# ---------------------------------------------------------------------------
# Custom fused DVE op (PSUM alpha stripe): one 1x pass that
#   - writes  out[p,k]     = min(in0[p,k], in1[p,k])     (column-min update)
#   - reduces accum_out[p] = min(s0[p], min_k in0[p,k])  (row-min of in0 ALONE)
# The stock Spec can only fold the *body* min(in0,in1) into accum_out, which
# would contaminate the row-min with column-accumulator values from other
# rows; repoint the accumulator's stream input to the raw Src0 delay chain.
# ---------------------------------------------------------------------------
_OP_NAME = "CHAMFER_COLROW"


def _chamfer_ref(in0, in1, c0, c1, c2):
    P = in0.shape[0]
    x = in0.astype(np.float32)
    body = np.minimum(x, in1.astype(np.float32))
    row = x.reshape(P, -1).min(axis=-1, keepdims=True)
    return body, np.minimum(c0, row)


def _register_chamfer_op():
    for op in _dve_ops.OPS:
        if op.name == _OP_NAME:
            return op
    spec = Spec(body=minn(Src0, Src1), accum=minn, accum_init=C0,
                reference=_chamfer_ref)
    op = _dve_ops.DveOp(_OP_NAME, spec, subdim=False, uops_sha={})
    taken = set(_dve_ops._SUB_OPCODE_FOR_NAME.values())
    row = next(r for r in free_opcode_rows("TRN2") if r not in taken)
    _dve_ops.OPS.append(op)
    _dve_ops.CUSTOM_DVE_SPECS[_OP_NAME] = spec
    _dve_ops._SUB_OPCODE_FOR_NAME[_OP_NAME] = row

    uops = _dve_lower(spec, ver="v3")
    assert len(uops) == 2
    acc_blk = uops[1].datapath_config[1]
    assert acc_blk.alu_src0 == AluInp.CURR_ALU_OUT
    assert acc_blk.alu_src1 == AluInp.PREV_ALU_OUT
    acc_blk.alu_src1 = AluInp.PREV_DELAY_0  # fold raw Src0, not the body
    for u in uops:
        u.validate("v3")
    _dve_ops._COMPILE_CACHE[(_OP_NAME, "v3")] = DveOpSpec(
        name=_OP_NAME, opcode=row, uops=uops, rd1_en=True
    )
    return op


_CHAMFER_OP = _register_chamfer_op()


def _build_bass():
    nc = bacc.Bacc()
    a_d = nc.declare_dram_parameter("a", [KA, N], BF16, isOutput=False)
    b_d = nc.declare_dram_parameter("b", [KA, COLS], BF16, isOutput=False)
    id_d = nc.declare_dram_parameter("ident", [128, 128], F32, isOutput=False)
    idb_d = nc.declare_dram_parameter("ident_bf", [128, 128], BF16,
                                      isOutput=False)
    out_d = nc.declare_dram_parameter("out", [128, OUTW], F32, isOutput=True)

    mmin = mybir.AluOpType.min

    with TileContext(nc) as tc:
        with (
            tc.tile_pool(name="const", bufs=1) as cpool,
            tc.tile_pool(name="work", bufs=1) as wpool,
            tc.tile_pool(name="stage", bufs=5) as spool,
            tc.tile_pool(name="ps", bufs=2, space="PSUM") as ppool,
        ):
            a_sb = cpool.tile([KA, N], BF16)
            b_sb = cpool.tile([KA, COLS], BF16)
            ident = cpool.tile([128, 128], F32)
            ident_bf = cpool.tile([128, 128], BF16)
            # ident first: the PE's first instruction (the warm transpose)
            # waits on it; everything else queues behind the PE.
            nc.sync.dma_start(out=ident[:], in_=id_d[:])
            nc.sync.dma_start(out=ident_bf[:], in_=idb_d[:])
            nc.sync.dma_start(out=b_sb[:, 0:512], in_=b_d[:, 0:512])
            nc.sync.dma_start(out=a_sb[:, 0:512], in_=a_d[:, 0:512])
            nc.sync.dma_start(out=b_sb[:, 512:COLS], in_=b_d[:, 512:COLS])
            nc.sync.dma_start(out=a_sb[:, 512:GW], in_=a_d[:, 512:GW])
            for q in range(1, N // GW):
                sl = slice(q * GW, (q + 1) * GW)
                nc.sync.dma_start(out=a_sb[:, sl], in_=a_d[:, sl])

            # out buffer: rm_a [0:IB] | rm_s [IB:2IB] | colminT [2IB:]
            outb = wpool.tile([128, OUTW], F32)
            rm_a = outb[:, 0:IB]
            rm_s = outb[:, IB:2 * IB]
            colminT = outb[:, 2 * IB:2 * IB + NCHUNK]
            colacc_a = wpool.tile([128, JG * WA], F32)
            colacc_s = wpool.tile([128, SW], BF16)
            nc.gpsimd.memset(colacc_a[:], BIG)
            nc.gpsimd.memset(colacc_s[:], BIG)

            # Make the PE observe the ident DMA queue before the main loop
            # so the tail transposes don't need an extra sync wait.
            warm = ppool.tile([128, 128], F32, tag="ps")
            nc.tensor.transpose(warm[:], ident[:], ident[:])

            for ib in range(IB):
                lhsT = a_sb[:, ib * 128:(ib + 1) * 128]
                ps = []
                for g in range(JG):
                    pst = ppool.tile([128, GW], F32, tag="ps")
                    for q in range(NQ):
                        j0 = g * GW + q * 512
                        nc.tensor.matmul(
                            pst[:, q * 512:(q + 1) * 512],
                            lhsT,
                            b_sb[:, j0:j0 + 512],
                        )
                    ps.append(pst)
                stage = spool.tile([128, SW], BF16, tag="stage")
                rma = rm_a[:, ib:ib + 1]
                for g in range(JG):
                    # alpha stripe: fused col+row min straight from PSUM.
                    # high_priority pins the customs early in the DVE stream:
                    # left alone, the scheduler defers them behind TT/TS, and
                    # the PSUM-pool recycling edge (next ACT copy waits this
                    # buffer's previous consumers) then stalls ACT ~700ns per
                    # i-block.
                    with tc.high_priority():
                        nc.vector._custom_dve(
                            _CHAMFER_OP,
                            out=colacc_a[:, g * WA:(g + 1) * WA],
                            accum_out=rma,
                            in0=ps[g][:, 0:WA],
                            in1=colacc_a[:, g * WA:(g + 1) * WA],
                            s0=BIG if g == 0 else rma,
                        )
                    # staged stripe: ACT downcasts PSUM fp32 -> SBUF bf16
                    nc.scalar.copy(
                        stage[:, g * WS:(g + 1) * WS], ps[g][:, WA:GW]
                    )
                # col-fold at 2x (bf16 tensor_tensor min, in-place accum)
                nc.vector.tensor_tensor(
                    out=colacc_s[:], in0=stage[:], in1=colacc_s[:], op=mmin
                )
                # row-fold at 4x (tensor_scalar min vs +BIG, min-reduce
                # accum; out is an in-place no-op write)
                nc.vector.tensor_scalar(
                    out=stage[:], in0=stage[:], scalar1=BIG, scalar2=None,
                    op0=mmin, op1=mmin, accum_out=rm_s[:, ib:ib + 1],
                )

            # Tail: partition-min of the column accumulators. PE transposes
            # packed per PSUM buffer (one bank per transpose) + batched
            # free-axis min reduces. colacc_a is fp32 (fp32 ident/psum),
            # colacc_s is bf16 (bf16 ident/psum, 1024-col bank spacing).
            def tail_batch(src, first, cnt, ident_t, spacing, dt, out0):
                pst = ppool.tile([128, spacing * 4], dt, tag="ps", name="pst")
                for q in range(cnt):
                    nc.tensor.transpose(
                        pst[:, q * spacing:q * spacing + 128],
                        src[:, (first + q) * 128:(first + q + 1) * 128],
                        ident_t[:],
                    )
                pst3d = pst[:].rearrange(
                    "p (b r) -> p b r", b=4
                )[:, 0:cnt, 0:128]
                nc.vector.tensor_reduce(
                    out=colminT[:, out0:out0 + cnt],
                    in_=pst3d,
                    axis=mybir.AxisListType.X,
                    op=mmin,
                )

            tail_batch(colacc_a, 0, ACHUNKS, ident, 512, F32, 0)
            for t4 in range((SCHUNKS + 3) // 4):
                cnt = min(4, SCHUNKS - t4 * 4)
                tail_batch(
                    colacc_s, t4 * 4, cnt, ident_bf, 1024, BF16,
                    ACHUNKS + t4 * 4,
                )

            nc.sync.dma_start(out=out_d[:], in_=outb[:])

    nc.finalize()
    return nc


def _get_nc():
    global _NC_CACHE
    if _NC_CACHE is None:
        _NC_CACHE = _build_bass()
    return _NC_CACHE


def kernel(**inputs) -> np.ndarray:
    import ml_dtypes

    bf16 = ml_dtypes.bfloat16

    def _bf(x):
        return x.astype(bf16).astype(np.float32)

    pred = np.asarray(inputs["registration_pred"], dtype=np.float32)
    gt = np.asarray(inputs["registration_gt"], dtype=np.float32)
    coords = np.asarray(inputs["coords"], dtype=np.float32)

    gt_pc = coords + gt        # [B, 3, N]  rows (i)
    gen_pc = coords + pred     # [B, 3, N]  cols (j)
    n1 = np.sum(gt_pc * gt_pc, axis=1)    # [B, N]
    n2 = np.sum(gen_pc * gen_pc, axis=1)  # [B, N]
    ident = np.eye(128, dtype=np.float32)
    ident_bf = np.eye(128, dtype=np.float32).astype(bf16)

    # bf16 hi/lo split: dot(x,y) ~ xh.yh + xl.yh + xh.yl (xl.yl dropped,
    # ~1e-5 relative); norms split into three bf16 terms.
    ones = np.ones((3, N), np.float32)

    def _split3(v):  # [N] fp32 -> [3, N] bf16 triplet summing to ~v
        h = _bf(v)
        m = _bf(v - h)
        l = _bf(v - h - m)
        return np.stack([h, m, l])

    in_maps = []
    for core in range(N_CORES):
        bi, half = core // 2, core % 2
        sl = slice(half * COLS, (half + 1) * COLS)
        x = gt_pc[bi]                      # [3, N]  all rows on every core
        xh = _bf(x)
        xl = _bf(x - xh)
        a = np.concatenate(
            [xh, xl, xh, _split3(n1[bi]), ones], axis=0
        )                                   # [15, N]
        y = gen_pc[bi][:, sl]               # [3, COLS]  this core's columns
        yh = _bf(y)
        yl = _bf(y - yh)
        bb = np.concatenate(
            [-2.0 * yh, -2.0 * yh, -2.0 * yl, ones[:, :COLS],
             _split3(n2[bi][sl])], axis=0
        )                                   # [15, COLS]
        in_maps.append(
            {
                "a": np.ascontiguousarray(a).astype(bf16),
                "b": np.ascontiguousarray(bb).astype(bf16),
                "ident": ident,
                "ident_bf": ident_bf,
            }
        )

    nc = _get_nc()
    global LAST_RESULTS
    res = run_bass_kernel_spmd(
        nc, in_maps, core_ids=list(range(N_CORES)), trace=TRACE
    )
    LAST_RESULTS = res

    d1 = np.zeros(B, np.float32)
    d2 = np.zeros(B, np.float32)
    for bi in range(B):
        o0 = res.results[2 * bi]["out"]      # cols 0..4095
        o1 = res.results[2 * bi + 1]["out"]  # cols 4096..8191
        # row-mins: combine alpha/staged partials, then the two col-halves
        rm0 = np.minimum(o0[:, 0:IB], o0[:, IB:2 * IB]).T.reshape(N)
        rm1 = np.minimum(o1[:, 0:IB], o1[:, IB:2 * IB]).T.reshape(N)
        d1[bi] = np.minimum(rm0, rm1).sum(dtype=np.float32)
        # col-mins are complete per core: sum every transposed-chunk entry
        cv0 = o0[:, 2 * IB:2 * IB + NCHUNK]
        cv1 = o1[:, 2 * IB:2 * IB + NCHUNK]
        d2[bi] = cv0.sum(dtype=np.float32) + cv1.sum(dtype=np.float32)

    out = np.float32(d1.mean(dtype=np.float32) + d2.mean(dtype=np.float32))
    return np.asarray(out, dtype=np.float32)


# revision 10
# speedup vs baseline: 1.1701x; 1.1701x over previous
"""Chamfer distance kernel for Trainium2 (8 NeuronCores, Bass/Tile).

Problem: B=4 pairs of 3-D point clouds with N=8192 points each.
  gt_pc  = coords + registration_gt   (rows  i of the distance matrix)
  gen_pc = coords + registration_pred (cols  j of the distance matrix)
  out = mean_b sum_i min_j d2[b,i,j] + mean_b sum_j min_i d2[b,i,j]

Strategy
  - Sharding: 8 cores = 4 batches x 2 column-halves (all 8192 rows, 4096
    cols each). Col-mins complete per core; row-min partials are
    min-combined across the 2 sibling cores on the host.
  - On-device: one augmented K=15 bf16 matmul produces squared distances
    in PSUM fp32 (hi/lo-split bf16 coordinates; exact in the PE's fp32
    accumulator up to the dropped xl.yl term, ~1e-5 relative).
  - The old kernel pushed every distance element through the DVE once at
    1 elem/cycle (fused custom op, 294us). This version splits the work
    so DVE and ACT are BOTH saturated:
      * alpha stripe (128 of every 2048 cols): DVE custom fused op reads
        PSUM fp32 directly -- col-min + row-min in one 1x pass.
      * staged stripe (1920 cols/group): ACT copies PSUM->SBUF with
        fp32->bf16 downcast (1 elem/cycle on the otherwise-idle ACT);
        then the DVE runs in its 16-bit high-perf modes on the bf16 data:
        tensor_tensor min col-fold at 2x (2x_1p) and a tensor_scalar
        (op0=min vs +BIG, op1=min accum_out) row-min reduce at 4x (4x_2p).
    Per i-block (2 groups x 2048 cols): DVE = 2x258 + 2060 + 1060 =
    3636 ns, ACT = 2x1785 = 3570 ns -- balanced. 64 i-blocks = 233 us
    vs the old all-DVE 281 us. (GPSIMD/compute-DMA/fp8 paths were probed
    and rejected: TensorTensor does not encode on Pool, DMACopy supports
    neither min nor max in Copy or CCE mode.)
  - bf16 distances cost ~0.4% relative per element; the final scalar is a
    sum of mins, measured ~1e-3 relative error -- tolerance is 2e-2.
  - Tail: colacc partition-min via PE transposes (bf16 ident for the bf16
    colacc) packed 4 per PSUM buffer + batched free-axis min reduces.
"""

import numpy as np

import concourse.bass as bass
import concourse.mybir as mybir
from concourse import bacc
from concourse import dve_ops as _dve_ops
from concourse.dve_spec import Spec, Src0, Src1, C0, minn, lower as _dve_lower
from concourse.dve_uop import AluInp, DveOpSpec
from concourse.dve_table_gen import free_opcode_rows
from concourse.tile import TileContext
from concourse.bass_utils import run_bass_kernel_spmd

B = 4
N = 8192
N_CORES = 8
COLS = N // 2            # columns per core (column-sharded: all rows local)
IB = N // 128            # 64 i-blocks per core
GW = 2048                # PSUM group width (4 banks)
JG = COLS // GW          # 2 j-groups
NQ = GW // 512           # 4 matmuls per group
F32 = mybir.dt.float32
BF16 = mybir.dt.bfloat16
KA = 15                  # augmented contraction dim (bf16 hi/lo split)
BIG = 3.0e38
WA = 128                 # alpha stripe width per group (custom fused, fp32)
WS = GW - WA             # staged stripe width per group (bf16 2x/4x path)
SW = JG * WS             # staged width per i-block (one TT + one TS op)
ACHUNKS = JG * WA // 128         # colacc_a transpose chunks (2)
SCHUNKS = SW // 128              # colacc_s transpose chunks (30)
NCHUNK = ACHUNKS + SCHUNKS       # 32
OUTW = IB + IB + NCHUNK          # rm_a | rm_s | colminT

# set by test harness to collect a profile
TRACE = False
LAST_RESULTS = None

_NC_CACHE = None


# ---------------------------------------------------------------------------
# Custom fused DVE op (PSUM alpha stripe): one 1x pass that
#   - writes  out[p,k]     = min(in0[p,k], in1[p,k])     (column-min update)
#   - reduces accum_out[p] = min(s0[p], min_k in0[p,k])  (row-min of in0 ALONE)
# The stock Spec can only fold the *body* min(in0,in1) into accum_out, which
# would contaminate the row-min with column-accumulator values from other
# rows; repoint the accumulator's stream input to the raw Src0 delay chain.
# ---------------------------------------------------------------------------
_OP_NAME = "CHAMFER_COLROW"


def _chamfer_ref(in0, in1, c0, c1, c2):
    P = in0.shape[0]
    x = in0.astype(np.float32)
    body = np.minimum(x, in1.astype(np.float32))
    row = x.reshape(P, -1).min(axis=-1, keepdims=True)
    return body, np.minimum(c0, row)


def _register_chamfer_op():
    for op in _dve_ops.OPS:
        if op.name == _OP_NAME:
            return op
    spec = Spec(body=minn(Src0, Src1), accum=minn, accum_init=C0,
                reference=_chamfer_ref)
    op = _dve_ops.DveOp(_OP_NAME, spec, subdim=False, uops_sha={})
    taken = set(_dve_ops._SUB_OPCODE_FOR_NAME.values())
    row = next(r for r in free_opcode_rows("TRN2") if r not in taken)
    _dve_ops.OPS.append(op)
    _dve_ops.CUSTOM_DVE_SPECS[_OP_NAME] = spec
    _dve_ops._SUB_OPCODE_FOR_NAME[_OP_NAME] = row

    uops = _dve_lower(spec, ver="v3")
    assert len(uops) == 2
    acc_blk = uops[1].datapath_config[1]
    assert acc_blk.alu_src0 == AluInp.CURR_ALU_OUT
    assert acc_blk.alu_src1 == AluInp.PREV_ALU_OUT
    acc_blk.alu_src1 = AluInp.PREV_DELAY_0  # fold raw Src0, not the body
    for u in uops:
        u.validate("v3")
    _dve_ops._COMPILE_CACHE[(_OP_NAME, "v3")] = DveOpSpec(
        name=_OP_NAME, opcode=row, uops=uops, rd1_en=True
    )
    return op


_CHAMFER_OP = _register_chamfer_op()


def _build_bass():
    nc = bacc.Bacc()
    a_d = nc.declare_dram_parameter("a", [KA, N], BF16, isOutput=False)
    b_d = nc.declare_dram_parameter("b", [KA, COLS], BF16, isOutput=False)
    id_d = nc.declare_dram_parameter("ident", [128, 128], F32, isOutput=False)
    idb_d = nc.declare_dram_parameter("ident_bf", [128, 128], BF16,
                                      isOutput=False)
    out_d = nc.declare_dram_parameter("out", [128, OUTW], F32, isOutput=True)

    mmin = mybir.AluOpType.min

    with TileContext(nc) as tc:
        with (
            tc.tile_pool(name="const", bufs=1) as cpool,
            tc.tile_pool(name="work", bufs=1) as wpool,
            tc.tile_pool(name="stage", bufs=5) as spool,
            tc.tile_pool(name="ps", bufs=2, space="PSUM") as ppool,
        ):
            a_sb = cpool.tile([KA, N], BF16)
            b_sb = cpool.tile([KA, COLS], BF16)
            ident = cpool.tile([128, 128], F32)
            ident_bf = cpool.tile([128, 128], BF16)
            # ident first: the PE's first instruction (the warm transpose)
            # waits on it; everything else queues behind the PE.
            nc.sync.dma_start(out=ident[:], in_=id_d[:])
            nc.sync.dma_start(out=ident_bf[:], in_=idb_d[:])
            nc.sync.dma_start(out=b_sb[:, 0:512], in_=b_d[:, 0:512])
            nc.sync.dma_start(out=a_sb[:, 0:512], in_=a_d[:, 0:512])
            nc.sync.dma_start(out=b_sb[:, 512:COLS], in_=b_d[:, 512:COLS])
            nc.sync.dma_start(out=a_sb[:, 512:GW], in_=a_d[:, 512:GW])
            for q in range(1, N // GW):
                sl = slice(q * GW, (q + 1) * GW)
                nc.sync.dma_start(out=a_sb[:, sl], in_=a_d[:, sl])

            # out buffer: rm_a [0:IB] | rm_s [IB:2IB] | colminT [2IB:]
            outb = wpool.tile([128, OUTW], F32)
            rm_a = outb[:, 0:IB]
            rm_s = outb[:, IB:2 * IB]
            colminT = outb[:, 2 * IB:2 * IB + NCHUNK]
            colacc_a = wpool.tile([128, JG * WA], F32)
            colacc_s = wpool.tile([128, SW], BF16)
            nc.gpsimd.memset(colacc_a[:], BIG)
            nc.gpsimd.memset(colacc_s[:], BIG)

            # Make the PE observe the ident DMA queue before the main loop
            # so the tail transposes don't need an extra sync wait.
            warm = ppool.tile([128, 128], F32, tag="ps")
            nc.tensor.transpose(warm[:], ident[:], ident[:])

            for ib in range(IB):
                lhsT = a_sb[:, ib * 128:(ib + 1) * 128]
                ps = []
                for g in range(JG):
                    pst = ppool.tile([128, GW], F32, tag="ps")
                    for q in range(NQ):
                        j0 = g * GW + q * 512
                        nc.tensor.matmul(
                            pst[:, q * 512:(q + 1) * 512],
                            lhsT,
                            b_sb[:, j0:j0 + 512],
                        )
                    ps.append(pst)
                stage = spool.tile([128, SW], BF16, tag="stage")
                rma = rm_a[:, ib:ib + 1]
                # ACT copies are emitted BEFORE the DVE customs: the Tile
                # scheduler pins cross-engine order roughly to emission
                # order, and pinning the copies behind the customs was
                # measured to stall ACT ~700ns per i-block.
                for g in range(JG):
                    # staged stripe: ACT downcasts PSUM fp32 -> SBUF bf16
                    nc.scalar.copy(
                        stage[:, g * WS:(g + 1) * WS], ps[g][:, WA:GW]
                    )
                for g in range(JG):
                    # alpha stripe: fused col+row min straight from PSUM
                    nc.vector._custom_dve(
                        _CHAMFER_OP,
                        out=colacc_a[:, g * WA:(g + 1) * WA],
                        accum_out=rma,
                        in0=ps[g][:, 0:WA],
                        in1=colacc_a[:, g * WA:(g + 1) * WA],
                        s0=BIG if g == 0 else rma,
                    )
                # col-fold at 2x (bf16 tensor_tensor min, in-place accum)
                nc.vector.tensor_tensor(
                    out=colacc_s[:], in0=stage[:], in1=colacc_s[:], op=mmin
                )
                # row-fold at 4x (tensor_scalar min vs +BIG, min-reduce
                # accum; out is an in-place no-op write)
                nc.vector.tensor_scalar(
                    out=stage[:], in0=stage[:], scalar1=BIG, scalar2=None,
                    op0=mmin, op1=mmin, accum_out=rm_s[:, ib:ib + 1],
                )

            # Tail: partition-min of the column accumulators. PE transposes
            # packed per PSUM buffer (one bank per transpose) + batched
            # free-axis min reduces. colacc_a is fp32 (fp32 ident/psum),
            # colacc_s is bf16 (bf16 ident/psum, 1024-col bank spacing).
            def tail_batch(src, first, cnt, ident_t, spacing, dt, out0):
                pst = ppool.tile([128, spacing * 4], dt, tag="ps", name="pst")
                for q in range(cnt):
                    nc.tensor.transpose(
                        pst[:, q * spacing:q * spacing + 128],
                        src[:, (first + q) * 128:(first + q + 1) * 128],
                        ident_t[:],
                    )
                pst3d = pst[:].rearrange(
                    "p (b r) -> p b r", b=4
                )[:, 0:cnt, 0:128]
                nc.vector.tensor_reduce(
                    out=colminT[:, out0:out0 + cnt],
                    in_=pst3d,
                    axis=mybir.AxisListType.X,
                    op=mmin,
                )

            tail_batch(colacc_a, 0, ACHUNKS, ident, 512, F32, 0)
            for t4 in range((SCHUNKS + 3) // 4):
                cnt = min(4, SCHUNKS - t4 * 4)
                tail_batch(
                    colacc_s, t4 * 4, cnt, ident_bf, 1024, BF16,
                    ACHUNKS + t4 * 4,
                )

            nc.sync.dma_start(out=out_d[:], in_=outb[:])

    nc.finalize()
    return nc


def _get_nc():
    global _NC_CACHE
    if _NC_CACHE is None:
        _NC_CACHE = _build_bass()
    return _NC_CACHE


def kernel(**inputs) -> np.ndarray:
    import ml_dtypes

    bf16 = ml_dtypes.bfloat16

    def _bf(x):
        return x.astype(bf16).astype(np.float32)

    pred = np.asarray(inputs["registration_pred"], dtype=np.float32)
    gt = np.asarray(inputs["registration_gt"], dtype=np.float32)
    coords = np.asarray(inputs["coords"], dtype=np.float32)

    gt_pc = coords + gt        # [B, 3, N]  rows (i)
    gen_pc = coords + pred     # [B, 3, N]  cols (j)
    n1 = np.sum(gt_pc * gt_pc, axis=1)    # [B, N]
    n2 = np.sum(gen_pc * gen_pc, axis=1)  # [B, N]
    ident = np.eye(128, dtype=np.float32)
    ident_bf = np.eye(128, dtype=np.float32).astype(bf16)

    # bf16 hi/lo split: dot(x,y) ~ xh.yh + xl.yh + xh.yl (xl.yl dropped,
    # ~1e-5 relative); norms split into three bf16 terms.
    ones = np.ones((3, N), np.float32)

    def _split3(v):  # [N] fp32 -> [3, N] bf16 triplet summing to ~v
        h = _bf(v)
        m = _bf(v - h)
        l = _bf(v - h - m)
        return np.stack([h, m, l])

    in_maps = []
    for core in range(N_CORES):
        bi, half = core // 2, core % 2
        sl = slice(half * COLS, (half + 1) * COLS)
        x = gt_pc[bi]                      # [3, N]  all rows on every core
        xh = _bf(x)
        xl = _bf(x - xh)
        a = np.concatenate(
            [xh, xl, xh, _split3(n1[bi]), ones], axis=0
        )                                   # [15, N]
        y = gen_pc[bi][:, sl]               # [3, COLS]  this core's columns
        yh = _bf(y)
        yl = _bf(y - yh)
        bb = np.concatenate(
            [-2.0 * yh, -2.0 * yh, -2.0 * yl, ones[:, :COLS],
             _split3(n2[bi][sl])], axis=0
        )                                   # [15, COLS]
        in_maps.append(
            {
                "a": np.ascontiguousarray(a).astype(bf16),
                "b": np.ascontiguousarray(bb).astype(bf16),
                "ident": ident,
                "ident_bf": ident_bf,
            }
        )

    nc = _get_nc()
    global LAST_RESULTS
    res = run_bass_kernel_spmd(
        nc, in_maps, core_ids=list(range(N_CORES)), trace=TRACE
    )
    LAST_RESULTS = res

    d1 = np.zeros(B, np.float32)
    d2 = np.zeros(B, np.float32)
    for bi in range(B):
        o0 = res.results[2 * bi]["out"]      # cols 0..4095
        o1 = res.results[2 * bi + 1]["out"]  # cols 4096..8191
        # row-mins: combine alpha/staged partials, then the two col-halves
        rm0 = np.minimum(o0[:, 0:IB], o0[:, IB:2 * IB]).T.reshape(N)
        rm1 = np.minimum(o1[:, 0:IB], o1[:, IB:2 * IB]).T.reshape(N)
        d1[bi] = np.minimum(rm0, rm1).sum(dtype=np.float32)
        # col-mins are complete per core: sum every transposed-chunk entry
        cv0 = o0[:, 2 * IB:2 * IB + NCHUNK]
        cv1 = o1[:, 2 * IB:2 * IB + NCHUNK]
        d2[bi] = cv0.sum(dtype=np.float32) + cv1.sum(dtype=np.float32)

    out = np.float32(d1.mean(dtype=np.float32) + d2.mean(dtype=np.float32))
    return np.asarray(out, dtype=np.float32)
